# revision 7
# baseline (speedup 1.0000x reference)
"""Multi-head causal attention (B=4, T=2048, D=1024, H=16) on 8 trn2 NeuronCores.

Sharding: 8 cores = 4 batches x 2 head-groups (8 heads each), Megatron-style.
Each core computes, for its (batch b, head-group hg):
  - QKV projection restricted to its 512 head-group columns (from x[b]^T)
  - causal attention for its 8 heads (scores kept transposed: [key, query]
    so softmax denominators come free via an appended ones-column on V)
  - partial output projection ctx @ W_o[hg rows]  ->  [T, D]
Host side: shard/transpose inputs, then out[b] = partial[2b] + partial[2b+1] + b_o.

All matmuls run as float32r (fp32 operands truncated to fp22 on the PE,
full fp32 accumulate) which streams at 1 row/cycle for free-dim >= 256.
"""

import numpy as np

B, T, D = 4, 2048, 1024
H, DK = 16, 64
HG = 8            # heads per core
C = 512           # head-group width (HG * DK)
NI = 8            # contraction chunks over D
NDC = 4           # 128-row chunks of the head-group dim
NT = 16           # 128-token chunks over T
NQW = 4           # 512-query windows
VW = DK + 1       # V chunk width incl. ones column

_CACHE = {}


def _build_nc():
    import concourse.bacc as bacc
    import concourse.mybir as mybir
    import concourse.tile as tile

    f32 = mybir.dt.float32
    f32r = mybir.dt.float32r
    AF = mybir.ActivationFunctionType
    ALU = mybir.AluOpType

    nc = bacc.Bacc(
        "TRN2",
        target_bir_lowering=False,
        debug=False,
        enable_asserts=False,
        num_devices=8,
    )

    xT_d = nc.dram_tensor("xT", [D, T], f32r, kind="ExternalInput").ap()
    wq_d = nc.dram_tensor("wq", [D, C], f32r, kind="ExternalInput").ap()
    wk_d = nc.dram_tensor("wk", [D, C], f32r, kind="ExternalInput").ap()
    wv_d = nc.dram_tensor("wv", [D, C], f32r, kind="ExternalInput").ap()
    wo_d = nc.dram_tensor("wo", [C, D], f32r, kind="ExternalInput").ap()
    bq_d = nc.dram_tensor("bq", [128, NDC], f32, kind="ExternalInput").ap()
    bk_d = nc.dram_tensor("bk", [128, NDC], f32, kind="ExternalInput").ap()
    bv_d = nc.dram_tensor("bv", [1, C], f32r, kind="ExternalInput").ap()
    out_d = nc.dram_tensor("out", [T, D], f32, kind="ExternalOutput").ap()

    with tile.TileContext(nc) as tc, nc.allow_low_precision(
        reason="fp32r matmul operands; fp22 mantissa is ample for 2e-2 rel tol"
    ):
        from contextlib import ExitStack

        with ExitStack() as ctx:
            singles = ctx.enter_context(tc.tile_pool(name="singles", bufs=1))
            xw_pool = ctx.enter_context(tc.tile_pool(name="xw", bufs=2))
            qt_pool = ctx.enter_context(tc.tile_pool(name="qt", bufs=2))
            ctx_pool = ctx.enter_context(tc.tile_pool(name="ctxw", bufs=1))
            pt_pool = ctx.enter_context(tc.tile_pool(name="pt", bufs=3))
            rec_pool = ctx.enter_context(tc.tile_pool(name="rec", bufs=2))
            rep_pool = ctx.enter_context(tc.tile_pool(name="rep", bufs=2))
            ob_pool = ctx.enter_context(tc.tile_pool(name="ob", bufs=2))
            ps_gen = ctx.enter_context(
                tc.tile_pool(name="psgen", bufs=3, space="PSUM")
            )
            ps_s = ctx.enter_context(tc.tile_pool(name="pss", bufs=3, space="PSUM"))
            ps_o = ctx.enter_context(tc.tile_pool(name="pso", bufs=2, space="PSUM"))

            # ---- persistent tiles ----
            wq_sb = singles.tile([128, NI * C], f32r, tag="wq", name="wq_sb")
            wk_sb = singles.tile([128, NI * C], f32r, tag="wk", name="wk_sb")
            wv_sb = singles.tile([128, NI * C], f32r, tag="wv", name="wv_sb")
            wo_sb = singles.tile([128, NDC * D], f32r, tag="wo", name="wo_sb")
            bq_sb = singles.tile([128, NDC], f32, tag="bq", name="bq_sb")
            bk_sb = singles.tile([128, NDC], f32, tag="bk", name="bk_sb")
            bv_sb = singles.tile([1, C], f32r, tag="bv", name="bv_sb")
            ones128 = singles.tile([1, 128], f32r, tag="o128", name="ones128")
            ones64 = singles.tile([1, 64], f32r, tag="o64", name="ones64")
            kt_sb = singles.tile([128, NDC * T], f32r, tag="kt", name="kt_sb")
            v_sb = singles.tile([128, HG * NT * VW], f32r, tag="v", name="v_sb")
            masks = singles.tile([128, 4 * 512], f32, tag="mask", name="masks")
            ones_f32 = singles.tile([128, 128], f32, tag="of32", name="ones_f32")

            # ---- weight/bias loads ----
            nc.sync.dma_start(
                wq_sb.rearrange("p (a d) -> p a d", d=C),
                wq_d.rearrange("(a p) d -> p a d", p=128),
            )
            nc.sync.dma_start(
                wk_sb.rearrange("p (a d) -> p a d", d=C),
                wk_d.rearrange("(a p) d -> p a d", p=128),
            )
            nc.sync.dma_start(
                wv_sb.rearrange("p (a d) -> p a d", d=C),
                wv_d.rearrange("(a p) d -> p a d", p=128),
            )
            nc.sync.dma_start(
                wo_sb.rearrange("p (a e) -> p a e", e=D),
                wo_d.rearrange("(a p) e -> p a e", p=128),
            )
            nc.sync.dma_start(bq_sb[:], bq_d[:])
            nc.sync.dma_start(bk_sb[:], bk_d[:])
            nc.sync.dma_start(bv_sb[:], bv_d[:])

            # memset cannot write f32r tiles (no ISA encoding); build ones in
            # an f32 scratch and DVE-copy (which rounds) into the f32r tiles.
            nc.gpsimd.memset(ones_f32[:], 1.0)
            nc.vector.tensor_copy(ones128[:], ones_f32[0:1, :])
            nc.vector.tensor_copy(ones64[:], ones_f32[0:1, :64])
            # ones column (index 64) of every V chunk
            nc.vector.tensor_copy(
                v_sb.rearrange("p (x c) -> p x c", c=VW)[:, :, DK : DK + 1],
                ones_f32[:],
            )
            # multiplicative causal masks for the 4 diagonal sub-blocks:
            # mask_m[t, q] = 1 if t <= q - 128*m else 0
            nc.gpsimd.memset(masks[:], 1.0)
            for m in range(4):
                nc.gpsimd.affine_select(
                    out=masks[:, m * 512 : (m + 1) * 512],
                    in_=masks[:, m * 512 : (m + 1) * 512],
                    compare_op=ALU.is_ge,
                    fill=0.0,
                    base=-128 * m,
                    channel_multiplier=-1,
                    pattern=[[1, 512]],
                )

            for qw in range(NQW):
                qt = qt_pool.tile([128, NDC * 512], f32r, tag="qt", name=f"qt{qw}")

                # ---- stage 1: QKV projections for the two 256-col half-windows
                for hw_ in (2 * qw, 2 * qw + 1):
                    xw = xw_pool.tile([128, NI * 256], f32r, tag="xw", name=f"xw{hw_}")
                    nc.sync.dma_start(
                        xw.rearrange("p (a t) -> p a t", t=256),
                        xT_d[:, hw_ * 256 : (hw_ + 1) * 256].rearrange(
                            "(a p) t -> p a t", p=128
                        ),
                    )
                    for wsb, bsb, dst, dco in (
                        (wq_sb, bq_sb, qt, lambda dc: dc * 512 + (hw_ % 2) * 256),
                        (wk_sb, bk_sb, kt_sb, lambda dc: dc * T + hw_ * 256),
                    ):
                        for dc in range(NDC):
                            ps = ps_gen.tile(
                                [128, 256], f32, tag="gen", name="psqk"
                            )
                            for ic in range(NI):
                                nc.tensor.matmul(
                                    ps[:],
                                    wsb[
                                        :, ic * C + dc * 128 : ic * C + dc * 128 + 128
                                    ],
                                    xw[:, ic * 256 : (ic + 1) * 256],
                                    start=(ic == 0),
                                    stop=(ic == NI - 1),
                                )
                            col = dco(dc)
                            nc.scalar.activation(
                                dst[:, col : col + 256],
                                ps[:],
                                AF.Identity,
                                bias=bsb[:, dc : dc + 1],
                            )
                    for j in range(2):
                        kt = 2 * hw_ + j
                        ps = ps_gen.tile([128, 512], f32, tag="gen", name="psv")
                        for ic in range(NI):
                            nc.tensor.matmul(
                                ps[:],
                                xw[
                                    :, ic * 256 + j * 128 : ic * 256 + j * 128 + 128
                                ],
                                wv_sb[:, ic * C : (ic + 1) * C],
                                start=(ic == 0),
                                stop=False,
                            )
                        nc.tensor.matmul(
                            ps[:],
                            ones128[:],
                            bv_sb[:],
                            start=False,
                            stop=True,
                        )
                        nc.vector.tensor_copy(
                            v_sb.rearrange("p (h x) -> p h x", h=HG)[
                                :, :, kt * VW : kt * VW + DK
                            ],
                            ps.rearrange("p (h d) -> p h d", h=HG),
                        )

                # ---- stage 2: attention for this query window, all 8 heads
                ctxw = ctx_pool.tile(
                    [128, NDC * 512], f32r, tag="ctxw", name=f"ctxw{qw}"
                )
                nkt = 4 * (qw + 1)
                for h in range(HG):
                    bp = 64 * (h % 2)
                    hc = h // 2
                    ops = ps_o.tile([VW, 512], f32, tag="o", name="ops")
                    for kt in range(nkt):
                        ss = ps_s.tile([128, 512], f32, tag="s", name="ss")
                        nc.tensor.matmul(
                            ss[:],
                            kt_sb[
                                bp : bp + 64, hc * T + kt * 128 : hc * T + kt * 128 + 128
                            ],
                            qt[bp : bp + 64, hc * 512 : (hc + 1) * 512],
                            start=True,
                            stop=True,
                        )
                        pt = pt_pool.tile([128, 512], f32r, tag="pt", name="pt")
                        nc.scalar.activation(pt[:], ss[:], AF.Exp, scale=0.125)
                        if kt >= 4 * qw:
                            m = kt - 4 * qw
                            nc.vector.tensor_mul(
                                pt[:], pt[:], masks[:, m * 512 : (m + 1) * 512]
                            )
                        nc.tensor.matmul(
                            ops[:],
                            v_sb[:, (h * NT + kt) * VW : (h * NT + kt + 1) * VW],
                            pt[:],
                            start=(kt == 0),
                            stop=(kt == nkt - 1),
                        )
                    rec = rec_pool.tile([1, 512], f32r, tag="rec", name="rec")
                    nc.vector.reciprocal(rec[:], ops[DK : DK + 1, :])
                    rps = ps_s.tile([64, 512], f32, tag="s", name="rps")
                    nc.tensor.matmul(
                        rps[:],
                        ones64[:],
                        rec[:],
                        start=True,
                        stop=True,
                    )
                    rep = rep_pool.tile([64, 512], f32, tag="rep", name="rep")
                    nc.vector.tensor_copy(rep[:], rps[:])
                    nc.vector.tensor_mul(
                        ctxw[bp : bp + 64, hc * 512 : (hc + 1) * 512],
                        ops[0:DK, :],
                        rep[:],
                    )

                # ---- stage 3: partial output projection for this window
                for jj in range(4):
                    ob = ob_pool.tile([128, D], f32, tag="ob", name="ob")
                    for eh in range(2):
                        ps3 = ps_gen.tile([128, 512], f32, tag="gen", name="ps3")
                        for cc in range(NDC):
                            nc.tensor.matmul(
                                ps3[:],
                                ctxw[
                                    :, cc * 512 + jj * 128 : cc * 512 + jj * 128 + 128
                                ],
                                wo_sb[:, cc * D + eh * 512 : cc * D + eh * 512 + 512],
                                start=(cc == 0),
                                stop=(cc == NDC - 1),
                            )
                        nc.vector.tensor_copy(ob[:, eh * 512 : (eh + 1) * 512], ps3[:])
                    tch = 4 * qw + jj
                    nc.sync.dma_start(out_d[tch * 128 : (tch + 1) * 128, :], ob[:])

    nc.compile()
    return nc


def _get_nc():
    if "nc" not in _CACHE:
        _CACHE["nc"] = _build_nc()
    return _CACHE["nc"]


def make_in_maps(x, W_qkv, b_qkv):
    x = np.asarray(x, np.float32)
    W_qkv = np.asarray(W_qkv, np.float32)
    b_qkv = np.asarray(b_qkv, np.float32)
    in_maps = []
    for c in range(8):
        b, hg = divmod(c, 2)
        s = slice(hg * C, (hg + 1) * C)
        in_maps.append(
            {
                "xT": np.ascontiguousarray(x[b].T),
                "wq": np.ascontiguousarray(W_qkv[:, 0 * D :][:, s]),
                "wk": np.ascontiguousarray(W_qkv[:, 1 * D :][:, s]),
                "wv": np.ascontiguousarray(W_qkv[:, 2 * D :][:, s]),
                "wo": None,  # filled by caller (needs W_o)
                "bq": np.ascontiguousarray(
                    b_qkv[0 * D :][s].reshape(NDC, 128).T
                ),
                "bk": np.ascontiguousarray(
                    b_qkv[1 * D :][s].reshape(NDC, 128).T
                ),
                "bv": np.ascontiguousarray(b_qkv[2 * D :][s].reshape(1, C)),
            }
        )
    return in_maps


def run(x, W_qkv, b_qkv, W_o, b_o, trace=False):
    from concourse.bass_utils import run_bass_kernel_spmd

    nc = _get_nc()
    W_o = np.asarray(W_o, np.float32)
    b_o = np.asarray(b_o, np.float32)
    in_maps = make_in_maps(x, W_qkv, b_qkv)
    for c in range(8):
        hg = c % 2
        in_maps[c]["wo"] = np.ascontiguousarray(W_o[hg * C : (hg + 1) * C, :])
    res = run_bass_kernel_spmd(
        nc, in_maps, core_ids=list(range(8)), trace=trace
    )
    parts = [res.results[c]["out"] for c in range(8)]
    out = np.stack(
        [parts[2 * b] + parts[2 * b + 1] + b_o[None, :] for b in range(B)]
    ).astype(np.float32)
    return out, res


def kernel(x, W_qkv, b_qkv, W_o, b_o):
    out, _ = run(x, W_qkv, b_qkv, W_o, b_o, trace=False)
    return out


# revision 10
# speedup vs baseline: 1.1147x; 1.1147x over previous
"""Multi-head causal attention (B=4, T=2048, D=1024, H=16) on 8 trn2 NeuronCores.

Sharding: 8 cores = 4 batches x 2 head-groups (8 heads each), Megatron-style.
Each core computes, for its (batch b, head-group hg):
  - QKV projection restricted to its 512 head-group columns (from x[b]^T)
  - causal attention for its 8 heads (scores kept transposed: [key, query]
    so softmax denominators come free via an appended ones-column on V)
  - partial output projection ctx @ W_o[hg rows]  ->  [T, D]
Host side: shard/transpose inputs, then out[b] = partial[2b] + partial[2b+1] + b_o.

All matmuls run as float32r (fp32 operands truncated to fp22 on the PE,
full fp32 accumulate) which streams at 1 row/cycle for free-dim >= 256.
"""

import numpy as np

B, T, D = 4, 2048, 1024
H, DK = 16, 64
HG = 8            # heads per core
C = 512           # head-group width (HG * DK)
NI = 8            # contraction chunks over D
NDC = 4           # 128-row chunks of the head-group dim
NT = 16           # 128-token chunks over T
NQW = 4           # 512-query windows
VW = DK + 1       # V chunk width incl. ones column

_CACHE = {}


def _build_nc():
    import concourse.bacc as bacc
    import concourse.mybir as mybir
    import concourse.tile as tile

    f32 = mybir.dt.float32
    f32r = mybir.dt.float32r
    AF = mybir.ActivationFunctionType
    ALU = mybir.AluOpType

    nc = bacc.Bacc(
        "TRN2",
        target_bir_lowering=False,
        debug=False,
        enable_asserts=False,
        num_devices=8,
    )

    xT_d = nc.dram_tensor("xT", [D, T], f32r, kind="ExternalInput").ap()
    wq_d = nc.dram_tensor("wq", [D, C], f32r, kind="ExternalInput").ap()
    wk_d = nc.dram_tensor("wk", [D, C], f32r, kind="ExternalInput").ap()
    wv_d = nc.dram_tensor("wv", [D, C], f32r, kind="ExternalInput").ap()
    wo_d = nc.dram_tensor("wo", [C, D], f32r, kind="ExternalInput").ap()
    bq_d = nc.dram_tensor("bq", [128, NDC], f32, kind="ExternalInput").ap()
    bk_d = nc.dram_tensor("bk", [128, NDC], f32, kind="ExternalInput").ap()
    bv_d = nc.dram_tensor("bv", [1, C], f32r, kind="ExternalInput").ap()
    out_d = nc.dram_tensor("out", [T, D], f32, kind="ExternalOutput").ap()

    with tile.TileContext(nc) as tc, nc.allow_low_precision(
        reason="fp32r matmul operands; fp22 mantissa is ample for 2e-2 rel tol"
    ):
        from contextlib import ExitStack

        with ExitStack() as ctx:
            singles = ctx.enter_context(tc.tile_pool(name="singles", bufs=1))
            xw_pool = ctx.enter_context(tc.tile_pool(name="xw", bufs=2))
            qt_pool = ctx.enter_context(tc.tile_pool(name="qt", bufs=2))
            ctx_pool = ctx.enter_context(tc.tile_pool(name="ctxw", bufs=2))
            pt_pool = ctx.enter_context(tc.tile_pool(name="pt", bufs=4))
            rec_pool = ctx.enter_context(tc.tile_pool(name="rec", bufs=1))
            rep_pool = ctx.enter_context(tc.tile_pool(name="rep", bufs=1))
            ob_pool = ctx.enter_context(tc.tile_pool(name="ob", bufs=2))
            ps_gen = ctx.enter_context(
                tc.tile_pool(name="psgen", bufs=2, space="PSUM")
            )
            ps_s = ctx.enter_context(tc.tile_pool(name="pss", bufs=4, space="PSUM"))
            ps_o = ctx.enter_context(tc.tile_pool(name="pso", bufs=2, space="PSUM"))

            # ---- persistent tiles ----
            wq_sb = singles.tile([128, NI * C], f32r, tag="wq", name="wq_sb")
            wk_sb = singles.tile([128, NI * C], f32r, tag="wk", name="wk_sb")
            wv_sb = singles.tile([128, NI * C], f32r, tag="wv", name="wv_sb")
            wo_sb = singles.tile([128, NDC * D], f32r, tag="wo", name="wo_sb")
            bq_sb = singles.tile([128, NDC], f32, tag="bq", name="bq_sb")
            bk_sb = singles.tile([128, NDC], f32, tag="bk", name="bk_sb")
            bv_sb = singles.tile([1, C], f32r, tag="bv", name="bv_sb")
            ones128 = singles.tile([1, 128], f32r, tag="o128", name="ones128")
            ones64 = singles.tile([1, 64], f32r, tag="o64", name="ones64")
            kt_sb = singles.tile([128, NDC * T], f32r, tag="kt", name="kt_sb")
            v_sb = singles.tile([128, HG * NT * VW], f32r, tag="v", name="v_sb")
            masks = singles.tile([128, 4 * 512], f32, tag="mask", name="masks")
            ones_f32 = singles.tile([128, 128], f32, tag="of32", name="ones_f32")

            # ---- weight/bias loads ----
            nc.sync.dma_start(
                wq_sb.rearrange("p (a d) -> p a d", d=C),
                wq_d.rearrange("(a p) d -> p a d", p=128),
            )
            nc.sync.dma_start(
                wk_sb.rearrange("p (a d) -> p a d", d=C),
                wk_d.rearrange("(a p) d -> p a d", p=128),
            )
            nc.sync.dma_start(
                wv_sb.rearrange("p (a d) -> p a d", d=C),
                wv_d.rearrange("(a p) d -> p a d", p=128),
            )
            nc.sync.dma_start(
                wo_sb.rearrange("p (a e) -> p a e", e=D),
                wo_d.rearrange("(a p) e -> p a e", p=128),
            )
            nc.sync.dma_start(bq_sb[:], bq_d[:])
            nc.sync.dma_start(bk_sb[:], bk_d[:])
            nc.sync.dma_start(bv_sb[:], bv_d[:])

            # memset cannot write f32r tiles (no ISA encoding); build ones in
            # an f32 scratch and DVE-copy (which rounds) into the f32r tiles.
            nc.gpsimd.memset(ones_f32[:], 1.0)
            nc.vector.tensor_copy(ones128[:], ones_f32[0:1, :])
            nc.vector.tensor_copy(ones64[:], ones_f32[0:1, :64])
            # ones column (index 64) of every V chunk
            nc.vector.tensor_copy(
                v_sb.rearrange("p (x c) -> p x c", c=VW)[:, :, DK : DK + 1],
                ones_f32[:],
            )
            # multiplicative causal masks for the 4 diagonal sub-blocks:
            # mask_m[t, q] = 1 if t <= q - 128*m else 0
            nc.gpsimd.memset(masks[:], 1.0)
            for m in range(4):
                nc.gpsimd.affine_select(
                    out=masks[:, m * 512 : (m + 1) * 512],
                    in_=masks[:, m * 512 : (m + 1) * 512],
                    compare_op=ALU.is_ge,
                    fill=0.0,
                    base=-128 * m,
                    channel_multiplier=-1,
                    pattern=[[1, 512]],
                )

            for qw in range(NQW):
                qt = qt_pool.tile([128, NDC * 512], f32r, tag="qt", name=f"qt{qw}")

                # ---- stage 1: QKV projections for the two 256-col half-windows
                for hw_ in (2 * qw, 2 * qw + 1):
                    xw = xw_pool.tile([128, NI * 256], f32r, tag="xw", name=f"xw{hw_}")
                    nc.sync.dma_start(
                        xw.rearrange("p (a t) -> p a t", t=256),
                        xT_d[:, hw_ * 256 : (hw_ + 1) * 256].rearrange(
                            "(a p) t -> p a t", p=128
                        ),
                    )
                    for wsb, bsb, dst, dco in (
                        (wq_sb, bq_sb, qt, lambda dc: dc * 512 + (hw_ % 2) * 256),
                        (wk_sb, bk_sb, kt_sb, lambda dc: dc * T + hw_ * 256),
                    ):
                        for dc in range(NDC):
                            ps = ps_gen.tile(
                                [128, 256], f32, tag="gen", name="psqk"
                            )
                            for ic in range(NI):
                                nc.tensor.matmul(
                                    ps[:],
                                    wsb[
                                        :, ic * C + dc * 128 : ic * C + dc * 128 + 128
                                    ],
                                    xw[:, ic * 256 : (ic + 1) * 256],
                                    start=(ic == 0),
                                    stop=(ic == NI - 1),
                                )
                            col = dco(dc)
                            nc.scalar.activation(
                                dst[:, col : col + 256],
                                ps[:],
                                AF.Identity,
                                bias=bsb[:, dc : dc + 1],
                            )
                    for j in range(2):
                        kt = 2 * hw_ + j
                        ps = ps_gen.tile([128, 512], f32, tag="gen", name="psv")
                        for ic in range(NI):
                            nc.tensor.matmul(
                                ps[:],
                                xw[
                                    :, ic * 256 + j * 128 : ic * 256 + j * 128 + 128
                                ],
                                wv_sb[:, ic * C : (ic + 1) * C],
                                start=(ic == 0),
                                stop=False,
                            )
                        nc.tensor.matmul(
                            ps[:],
                            ones128[:],
                            bv_sb[:],
                            start=False,
                            stop=True,
                        )
                        nc.vector.tensor_copy(
                            v_sb.rearrange("p (h x) -> p h x", h=HG)[
                                :, :, kt * VW : kt * VW + DK
                            ],
                            ps.rearrange("p (h d) -> p h d", h=HG),
                        )

                # ---- stage 2: attention for this query window, all 8 heads
                ctxw = ctx_pool.tile(
                    [128, NDC * 512], f32r, tag="ctxw", name=f"ctxw{qw}"
                )
                # heads processed in pairs: the two K=64 S-matmuls live in
                # different PE row-groups (base partition 0 / 64) and the two
                # S->exp->AV chains are independent, which keeps the PE fed
                # (and HAM warm) while ACT computes the exp of earlier blocks.
                nkt = 4 * (qw + 1)
                for hp in range(HG // 2):
                    hc = hp  # column block == pair index
                    ops_pair = [
                        ps_o.tile([VW, 512], f32, tag="o", name=f"ops{h}")
                        for h in (0, 1)
                    ]
                    for kt in range(nkt):
                        # narrow diagonal blocks: only q >= kt*128 is valid
                        m = kt - 4 * qw
                        off = min(128 * m, 256) if m > 0 else 0
                        w = 512 - off
                        pts = []
                        for i in (0, 1):
                            bp = 64 * i
                            ss = ps_s.tile([128, 512], f32, tag="s", name="ss")
                            nc.tensor.matmul(
                                ss[:, off:],
                                kt_sb[
                                    bp : bp + 64,
                                    hc * T + kt * 128 : hc * T + kt * 128 + 128,
                                ],
                                qt[bp : bp + 64, hc * 512 + off : (hc + 1) * 512],
                                start=True,
                                stop=True,
                            )
                            pt = pt_pool.tile([128, 512], f32r, tag="pt", name="pt")
                            nc.scalar.activation(
                                pt[:, off:], ss[:, off:], AF.Exp, scale=0.125
                            )
                            if m >= 0:
                                nc.vector.tensor_mul(
                                    pt[:, off:],
                                    pt[:, off:],
                                    masks[:, m * 512 + off : (m + 1) * 512],
                                )
                            pts.append(pt)
                        for i in (0, 1):
                            h = 2 * hp + i
                            nc.tensor.matmul(
                                ops_pair[i][:, off:],
                                v_sb[:, (h * NT + kt) * VW : (h * NT + kt + 1) * VW],
                                pts[i][:, off:],
                                start=(kt == 0),
                                stop=(kt == nkt - 1),
                            )
                    for i in (0, 1):
                        bp = 64 * i
                        ops = ops_pair[i]
                        rec = rec_pool.tile([1, 512], f32r, tag="rec", name="rec")
                        nc.vector.reciprocal(rec[:], ops[DK : DK + 1, :])
                        rps = ps_s.tile([64, 512], f32, tag="s", name="rps")
                        nc.tensor.matmul(
                            rps[:],
                            ones64[:],
                            rec[:],
                            start=True,
                            stop=True,
                        )
                        rep = rep_pool.tile([64, 512], f32, tag="rep", name="rep")
                        nc.vector.tensor_copy(rep[:], rps[:])
                        nc.vector.tensor_mul(
                            ctxw[bp : bp + 64, hc * 512 : (hc + 1) * 512],
                            ops[0:DK, :],
                            rep[:],
                        )

                # ---- stage 3: partial output projection for this window
                for jj in range(4):
                    ob = ob_pool.tile([128, D], f32, tag="ob", name="ob")
                    for eh in range(2):
                        ps3 = ps_gen.tile([128, 512], f32, tag="gen", name="ps3")
                        for cc in range(NDC):
                            nc.tensor.matmul(
                                ps3[:],
                                ctxw[
                                    :, cc * 512 + jj * 128 : cc * 512 + jj * 128 + 128
                                ],
                                wo_sb[:, cc * D + eh * 512 : cc * D + eh * 512 + 512],
                                start=(cc == 0),
                                stop=(cc == NDC - 1),
                            )
                        nc.vector.tensor_copy(ob[:, eh * 512 : (eh + 1) * 512], ps3[:])
                    tch = 4 * qw + jj
                    nc.sync.dma_start(out_d[tch * 128 : (tch + 1) * 128, :], ob[:])

    nc.compile()
    return nc


def _get_nc():
    if "nc" not in _CACHE:
        _CACHE["nc"] = _build_nc()
    return _CACHE["nc"]


def make_in_maps(x, W_qkv, b_qkv):
    x = np.asarray(x, np.float32)
    W_qkv = np.asarray(W_qkv, np.float32)
    b_qkv = np.asarray(b_qkv, np.float32)
    in_maps = []
    for c in range(8):
        b, hg = divmod(c, 2)
        s = slice(hg * C, (hg + 1) * C)
        in_maps.append(
            {
                "xT": np.ascontiguousarray(x[b].T),
                "wq": np.ascontiguousarray(W_qkv[:, 0 * D :][:, s]),
                "wk": np.ascontiguousarray(W_qkv[:, 1 * D :][:, s]),
                "wv": np.ascontiguousarray(W_qkv[:, 2 * D :][:, s]),
                "wo": None,  # filled by caller (needs W_o)
                "bq": np.ascontiguousarray(
                    b_qkv[0 * D :][s].reshape(NDC, 128).T
                ),
                "bk": np.ascontiguousarray(
                    b_qkv[1 * D :][s].reshape(NDC, 128).T
                ),
                "bv": np.ascontiguousarray(b_qkv[2 * D :][s].reshape(1, C)),
            }
        )
    return in_maps


def run(x, W_qkv, b_qkv, W_o, b_o, trace=False):
    from concourse.bass_utils import run_bass_kernel_spmd

    nc = _get_nc()
    W_o = np.asarray(W_o, np.float32)
    b_o = np.asarray(b_o, np.float32)
    in_maps = make_in_maps(x, W_qkv, b_qkv)
    for c in range(8):
        hg = c % 2
        in_maps[c]["wo"] = np.ascontiguousarray(W_o[hg * C : (hg + 1) * C, :])
    res = run_bass_kernel_spmd(
        nc, in_maps, core_ids=list(range(8)), trace=trace
    )
    parts = [res.results[c]["out"] for c in range(8)]
    out = np.stack(
        [parts[2 * b] + parts[2 * b + 1] + b_o[None, :] for b in range(B)]
    ).astype(np.float32)
    return out, res


def kernel(x, W_qkv, b_qkv, W_o, b_o):
    out, _ = run(x, W_qkv, b_qkv, W_o, b_o, trace=False)
    return out
